# revision 1
# baseline (speedup 1.0000x reference)
"""Causal self-attention Trainium2 kernel.

B=4, T=2048, C=1024, H=16 heads, D=64. 8 NeuronCores, tensor-parallel over
heads: core c owns heads {2c, 2c+1}. Host pre-transposes x to xT [C, B*T],
column-shards W_attn / row-shards W_proj, sums the 8 partial outputs.

Device kernel (per core, SPMD), software-pipelined one batch deep so the
PE-heavy qkv matmuls fill the gaps of the ACT-paced attention stream:
  qkv:  qkvT[384, T] = W_core.T @ xT  (bf16 matmuls, K=C in 8 chunks);
        rows: [qA qB | kA kB | vA vB], 64 each. Bias added on eviction (DVE).
        v rows are PE-transposed to token-major V with 64 appended ones
        columns, so the y^T matmul emits the softmax denominators
        replicated on psum partitions 64-127, row-aligned with y.
  attn: S^T layout: S^T[keys,queries] = k @ q^T via matmul(lhsT=kT_chunk,
        rhs=qT_block, fp32r); the two heads' K=64 matmuls are issued
        adjacently so the PE can run them concurrently in different row
        groups. exp on ACT over both heads at once ([128, 2, 512] psum);
        causal mask via per-head column-restricted gpsimd affine_select;
        diagonal chunks restrict all work to the live query range.
        y^T[d, queries] accumulated via matmul(lhsT=[V|1s], rhs=P^T, bf16);
        normalization is recip + row-aligned mul on DVE.
  proj: partial out[tokens, C] = y^T.T @ W_proj_rows (fp32r), DMA'd out;
        deferred one block so the PE has ready work during normalize.
"""

import sys

sys.path.insert(0, "/opt/trn_rl_repo")

from contextlib import ExitStack

import numpy as np

import concourse.bass as bass
import concourse.mybir as mybir
import concourse.tile as tile
from concourse import bacc
from concourse.bass_utils import run_bass_kernel_spmd
from concourse.masks import make_identity

F32 = mybir.dt.float32
F32R = mybir.dt.float32r
BF16 = mybir.dt.bfloat16
AF = mybir.ActivationFunctionType

B, T, C, H, D = 4, 2048, 1024, 16, 64
NCORES = 8
HPC = H // NCORES  # heads per core = 2
TOK = B * T  # 8192
QKVC = HPC * D  # per-core channels per q/k/v = 128
TB = 256  # token block for the qkv phase
NBB = T // TB  # qkv token blocks per batch = 8
QB = 512  # query block for attention
NKC = T // 128  # key chunks per batch = 16
SCALE = 1.0 / 8.0  # 1/sqrt(D)


def build_program():
    nc = bacc.Bacc(
        "TRN2",
        target_bir_lowering=False,
        debug=False,
        num_devices=NCORES,
    )
    xt_d = nc.dram_tensor("xt", [C, TOK], BF16, kind="ExternalInput").ap()
    wqkv_d = nc.dram_tensor("wqkv", [C, 3 * QKVC], BF16, kind="ExternalInput").ap()
    bqkv_d = nc.dram_tensor("bqkv", [3 * QKVC], F32, kind="ExternalInput").ap()
    wproj_d = nc.dram_tensor("wproj", [QKVC, C], F32R, kind="ExternalInput").ap()
    outp_d = nc.dram_tensor("outp", [TOK, C], F32, kind="ExternalOutput").ap()

    with tile.TileContext(nc) as tc:
        with ExitStack() as ctx, nc.allow_low_precision(reason="fp32r matmul inputs"):
            _body(ctx, tc, xt_d, wqkv_d, bqkv_d, wproj_d, outp_d)
    nc.compile()
    return nc


class _Kern:
    def __init__(self, ctx, tc, xt_d, wqkv_d, bqkv_d, wproj_d, outp_d):
        nc = tc.nc
        self.nc = nc
        self.tc = tc
        self.outp_d = outp_d

        self.const = ctx.enter_context(tc.tile_pool(name="const", bufs=1))
        self.persist = ctx.enter_context(tc.tile_pool(name="persist", bufs=1))
        self.xt_pool = ctx.enter_context(tc.tile_pool(name="xt", bufs=3))
        self.vtmp_pool = ctx.enter_context(tc.tile_pool(name="vtmp", bufs=3))
        self.pt_pool = ctx.enter_context(tc.tile_pool(name="pt", bufs=8))
        self.yt_pool = ctx.enter_context(tc.tile_pool(name="yt", bufs=3))
        self.out_pool = ctx.enter_context(tc.tile_pool(name="osb", bufs=4))
        self.small_pool = ctx.enter_context(tc.tile_pool(name="small", bufs=4))

        self.ps_s = ctx.enter_context(tc.tile_pool(name="ps_s", bufs=2, space="PSUM"))
        self.ps_y = ctx.enter_context(tc.tile_pool(name="ps_y", bufs=2, space="PSUM"))
        self.ps_mm = ctx.enter_context(tc.tile_pool(name="ps_mm", bufs=2, space="PSUM"))

        # --- constants ---
        c = self.const
        # weight loads ride the ACT HWDGE ring (nc.scalar) so they don't
        # serialize with the xt streaming loads on the SP ring; the first
        # K-chunk is split out so the PE can start quickly
        self.wqkv_s = c.tile([128, 8, 3 * QKVC], BF16, tag="wqkv", name="wqkv_s")
        wqkv_r = wqkv_d.rearrange("(kc p) m -> p kc m", p=128)
        for kc in range(8):
            nc.scalar.dma_start(self.wqkv_s[:, kc : kc + 1, :], wqkv_r[:, kc : kc + 1, :])
        self.bqkv_s = c.tile([128, 3], F32, tag="bqkv", name="bqkv_s")
        nc.scalar.dma_start(self.bqkv_s[:], bqkv_d.rearrange("(m p) -> p m", p=128))
        self.wproj_s = c.tile([128, C], F32R, tag="wproj", name="wproj_s")
        nc.scalar.dma_start(self.wproj_s[:], wproj_d[:])
        self.ident = c.tile([128, 128], F32, tag="ident", name="ident")
        make_identity(nc, self.ident[:])

        # persistent activations
        self.qT = self.persist.tile([128, TOK], F32R, tag="qT", name="qT")
        self.kT = self.persist.tile([128, TOK], F32R, tag="kT", name="kT")
        # token-major V (cols 0:D) + 64 replicated ones columns (cols D:2D):
        # the y^T matmul then yields the softmax denominator replicated on
        # psum partitions D..2D, row-aligned with y for the normalize mul
        self.vones = self.persist.tile(
            [128, B, HPC, NKC, 2 * D], BF16, tag="vones", name="vones"
        )
        nc.gpsimd.memset(self.vones[:, :, :, :, D : 2 * D], 1.0)
        self.xt_r = xt_d.rearrange("(kc p) t -> p kc t", p=128)

    def qkv_block(self, b, nb):
        """QKV + V-transpose for token block nb (TB tokens) of batch b."""
        nc = self.nc
        n = b * NBB + nb
        xt_t = self.xt_pool.tile([128, 8, TB], BF16, tag="xt", name=f"xt{n}")
        if n == 0:
            # cold start: split the first load so the PE can start sooner
            for kc in range(8):
                nc.sync.dma_start(
                    xt_t[:, kc, :], self.xt_r[:, kc, n * TB : (n + 1) * TB]
                )
        else:
            nc.sync.dma_start(xt_t[:], self.xt_r[:, :, n * TB : (n + 1) * TB])
        for m in range(3):  # q, k, v row chunks
            ps = self.ps_mm.tile([128, TB], F32, tag="mm", name=f"qkvp{n}_{m}")
            for kc in range(8):
                nc.tensor.matmul(
                    ps[:],
                    self.wqkv_s[:, kc, m * 128 : (m + 1) * 128],
                    xt_t[:, kc, :],
                    start=(kc == 0),
                    stop=(kc == 7),
                )
            if m < 2:
                dst = (self.qT if m == 0 else self.kT)[:, n * TB : (n + 1) * TB]
                nc.vector.tensor_scalar_add(dst, ps[:], self.bqkv_s[:, m : m + 1])
            else:
                vt = self.vtmp_pool.tile([128, TB], F32, tag="vt", name=f"vt{n}")
                nc.vector.tensor_scalar_add(vt[:], ps[:], self.bqkv_s[:, 2:3])
                j0 = (TB // 128) * nb
                for jj in range(TB // 128):
                    pst = self.ps_mm.tile([128, 128], F32, tag="mm", name=f"tr{n}_{jj}")
                    nc.tensor.transpose(
                        pst[:], vt[:, jj * 128 : (jj + 1) * 128], self.ident[:]
                    )
                    nc.vector.tensor_copy(
                        self.vones[:, b, :, j0 + jj, 0:D],
                        pst[:].rearrange("p (h d) -> p h d", h=HPC),
                    )

    def attn_block(self, b, qb):
        """Attention + proj for query block qb (QB queries) of batch b."""
        nc = self.nc
        q0 = b * T + qb * QB
        nj = (qb + 1) * (QB // 128)  # key chunks attended by this block
        psy = [
            self.ps_y.tile([2 * D, QB], F32, tag="psy", name=f"psy{b}_{qb}_{h}")
            for h in range(HPC)
        ]
        for j in range(nj):  # key chunks of 128
            k0 = b * T + j * 128
            # diagonal trimming: for a diagonal chunk at offset d, queries
            # f < 128*d attend to no key in this chunk, so restrict all work
            # to the query range [f0, QB)
            d = j - (nj - 4)
            # cap the restriction at 256 live queries: below that, fp32r
            # matmuls drop to 4 cyc/row and the "saved" columns cost more
            # than computing them (the mask zeroes them regardless)
            f0 = min(128 * d, QB - 256) if d > 0 else 0
            # one 2-bank psum tile holds both heads' S^T for this chunk;
            # the two K=64 matmuls use partitions 0-63 / 64-127 -> different
            # PE row groups, issued adjacently so they can run concurrently
            ps2 = self.ps_s.tile([128, HPC, QB], F32, tag="s2", name=f"s{b}_{qb}_{j}")
            for h in range(HPC):
                nc.tensor.matmul(
                    ps2[:, h, f0:QB],
                    self.kT[h * D : (h + 1) * D, k0 : k0 + 128],
                    self.qT[h * D : (h + 1) * D, q0 + f0 : q0 + QB],
                    start=True,
                    stop=True,
                )
            pt = self.pt_pool.tile([128, HPC, QB], BF16, tag="pt", name=f"pt{b}_{qb}_{j}")
            nc.scalar.activation(pt[:, :, f0:QB], ps2[:, :, f0:QB], AF.Exp, scale=SCALE)
            if d >= 0:
                # mask only the 128-column window straddling the diagonal,
                # per head so the first yT matmul isn't gated on both
                cols = min(QB, 128 * (d + 1))
                for h in range(HPC):
                    nc.gpsimd.affine_select(
                        out=pt[:, h, f0:cols],
                        in_=pt[:, h, f0:cols],
                        base=QB * qb - 128 * j + f0,
                        channel_multiplier=-1,
                        pattern=[[1, cols - f0]],
                        compare_op=mybir.AluOpType.is_ge,
                        fill=0.0,
                    )
            for h in range(HPC):
                nc.tensor.matmul(
                    psy[h][:, f0:QB],
                    self.vones[:, b, h, j, :],
                    pt[:, h, f0:QB],
                    start=(j == 0),
                    stop=(j == nj - 1),
                )
        # normalize into yt (d-major, both heads stacked)
        yt = self.yt_pool.tile([128, QB], F32R, tag="yt", name=f"yt{b}_{qb}")
        for h in range(HPC):
            rec = self.small_pool.tile([D, QB], F32, tag="rec", name=f"rec{b}_{qb}_{h}")
            nc.vector.reciprocal(rec[:], psy[h][D : 2 * D, :])
            nc.vector.tensor_mul(yt[h * D : (h + 1) * D, :], psy[h][0:D, :], rec[:])
        return yt

    def proj_block(self, b, qb, yt):
        """Projection + output DMA for query block qb of batch b."""
        nc = self.nc
        q0 = b * T + qb * QB
        for tt in range(QB // 128):
            osb = self.out_pool.tile([128, C], F32, tag="osb", name=f"o{b}_{qb}_{tt}")
            for ncol in range(C // 512):
                po = self.ps_mm.tile([128, 512], F32, tag="mm", name=f"po{b}_{qb}_{tt}_{ncol}")
                nc.tensor.matmul(
                    po[:],
                    yt[:, tt * 128 : (tt + 1) * 128],
                    self.wproj_s[:, ncol * 512 : (ncol + 1) * 512],
                    start=True,
                    stop=True,
                )
                nc.vector.tensor_copy(osb[:, ncol * 512 : (ncol + 1) * 512], po[:])
            r0 = q0 + tt * 128
            nc.sync.dma_start(self.outp_d[r0 : r0 + 128, :], osb[:])


def _body(ctx, tc, xt_d, wqkv_d, bqkv_d, wproj_d, outp_d):
    k = _Kern(ctx, tc, xt_d, wqkv_d, bqkv_d, wproj_d, outp_d)
    # Software pipeline one batch deep: attention(b) interleaves with the
    # independent qkv(b+1) blocks so the PE always has ready matmuls while
    # ACT paces the softmax. proj is deferred one attention block so the PE
    # has ready work while the softmax-normalize chain completes.
    # qkv(0) is the prologue; batches 1..B-1 form a queue drained 2 blocks
    # per attention slot for the first half, then 1, so every attention
    # stretch (including the last batch's) has PE-dense qkv filler.
    pending = None
    for nb in range(NBB):
        k.qkv_block(0, nb)
    queue = [(b, nb) for b in range(1, B) for nb in range(NBB)]
    qi = 0
    nslots = B * (T // QB)
    for s in range(nslots):
        b, qb = s // (T // QB), s % (T // QB)
        want = 2 if s < nslots // 2 else 1
        # never emit attn before its qkv blocks: need batch b block 2qb+1
        need = 0 if b == 0 else (b - 1) * NBB + 2 * qb + 2
        while qi < len(queue) and (qi < need or want > 0):
            k.qkv_block(*queue[qi])
            qi += 1
            want -= 1
        yt = k.attn_block(b, qb)
        if pending is not None:
            k.proj_block(*pending)
        pending = (b, qb, yt)
    while qi < len(queue):
        k.qkv_block(*queue[qi])
        qi += 1
    k.proj_block(*pending)


_CACHED_NC = None


def _get_nc():
    global _CACHED_NC
    if _CACHED_NC is None:
        _CACHED_NC = build_program()
    return _CACHED_NC


def make_in_maps(x, W_attn, b_attn, W_proj):
    x = np.ascontiguousarray(np.asarray(x, dtype=np.float32))
    W_attn = np.asarray(W_attn, dtype=np.float32)
    b_attn = np.asarray(b_attn, dtype=np.float32)
    W_proj = np.asarray(W_proj, dtype=np.float32)
    import ml_dtypes

    xt = np.ascontiguousarray(x.reshape(TOK, C).T.astype(ml_dtypes.bfloat16))
    in_maps = []
    for c in range(NCORES):
        s = c * QKVC
        wq = W_attn[:, s : s + QKVC]
        wk = W_attn[:, C + s : C + s + QKVC]
        wv = W_attn[:, 2 * C + s : 2 * C + s + QKVC]
        wqkv = np.ascontiguousarray(
            np.concatenate([wq, wk, wv], axis=1).astype(ml_dtypes.bfloat16)
        )
        bq = b_attn[s : s + QKVC]
        bk = b_attn[C + s : C + s + QKVC]
        bv = b_attn[2 * C + s : 2 * C + s + QKVC]
        bqkv = np.ascontiguousarray(np.concatenate([bq, bk, bv]))
        wproj = np.ascontiguousarray(W_proj[s : s + QKVC, :])
        in_maps.append({"xt": xt, "wqkv": wqkv, "bqkv": bqkv, "wproj": wproj})
    return in_maps


def run(x, W_attn, b_attn, W_proj, b_proj, trace=False, **kwargs):
    nc = _get_nc()
    in_maps = make_in_maps(x, W_attn, b_attn, W_proj)
    res = run_bass_kernel_spmd(
        nc, in_maps, core_ids=list(range(NCORES)), trace=trace, **kwargs
    )
    acc = res.results[0]["outp"].astype(np.float32, copy=True)
    for c in range(1, NCORES):
        acc += res.results[c]["outp"]
    acc += np.asarray(b_proj, dtype=np.float32)[None, :]
    out = acc.reshape(B, T, C)
    return out, res


def kernel(x, W_attn, b_attn, W_proj, b_proj):
    out, _ = run(x, W_attn, b_attn, W_proj, b_proj, trace=False)
    return out



# revision 19
# speedup vs baseline: 1.2241x; 1.2241x over previous
"""Causal self-attention Trainium2 kernel.

B=4, T=2048, C=1024, H=16 heads, D=64. 8 NeuronCores, sharded
batch x head-half: core (b, g) owns batch b and heads [8g, 8g+8).
Host pre-transposes x[b] to xT8 [C, T] fp8, packs DoubleRow weight
layouts, sums the per-batch core pairs' partial proj outputs.

Device kernel (per core, SPMD):
  qkv:  q,k d-major via fp8 DoubleRow matmuls (K=2x128 per instr):
        psum chunk c = [head-c q d0:64 | k d0:64] rows; evicted (DVE,
        per-partition bias, bf16) to qkT [128, 8, T].
        v token-major (lhsT = xT chunk): psum [128 tok, 512]; evicted
        (DVE, +bias replicated, fp8) into vT8 [tok, jp, i, h, v|ones],
        the [v64|ones64] columns make the PV matmul emit softmax
        denominators on psum partitions 64:128.
  attn: per (h, qb): S^T[keys, queries] bf16 matmuls per 128-key chunk,
        processed in chunk PAIRS: exp on ACT over [128, 2, cols] psum
        -> pt fp8; causal mask via gpsimd affine_select on diagonal
        chunks; PV as ONE fp8 DoubleRow matmul per pair (contracts 256
        keys) accumulating psy [y64|den64, cols].
        normalize: DVE reciprocal (partition-shifted) + mul -> yt f32r.
  proj: partial out[tok, C] = yt.T @ W_proj rows (fp32r), evicted via
        gpsimd copy, DMA'd out; deferred one block for PE slack.
"""

import sys

sys.path.insert(0, "/opt/trn_rl_repo")

from contextlib import ExitStack

import numpy as np

import concourse.bass as bass
import concourse.mybir as mybir
import concourse.tile as tile
from concourse import bacc
from concourse.bass_utils import run_bass_kernel_spmd

F32 = mybir.dt.float32
F32R = mybir.dt.float32r
BF16 = mybir.dt.bfloat16
FP8 = mybir.dt.float8e4
AF = mybir.ActivationFunctionType
DR = mybir.MatmulPerfMode.DoubleRow

B, T, C, H, D = 4, 2048, 1024, 16, 64
NCORES = 8
HPC = H // 2  # heads per core = 8
TB = 512  # token block for qkv
NTB = T // TB  # 4
QB = 512  # query block
NQB = T // QB  # 4
SCALE = 1.0 / 8.0  # 1/sqrt(D)


def build_program():
    nc = bacc.Bacc(
        "TRN2",
        target_bir_lowering=False,
        debug=False,
        num_devices=NCORES,
    )
    d = {}
    d["xt"] = nc.dram_tensor("xt", [128, 8, T], FP8, kind="ExternalInput").ap()
    d["xt16"] = nc.dram_tensor("xt16", [128, 8, TB], BF16, kind="ExternalInput").ap()
    d["wqk"] = nc.dram_tensor("wqk", [128, 4, 2, 8, 128], FP8, kind="ExternalInput").ap()
    d["wqk16"] = nc.dram_tensor(
        "wqk16", [128, 8, 8, 128], BF16, kind="ExternalInput"
    ).ap()
    d["wv"] = nc.dram_tensor("wv", [128, 4, 2, 512], FP8, kind="ExternalInput").ap()
    d["wv16"] = nc.dram_tensor("wv16", [128, 8, 512], BF16, kind="ExternalInput").ap()
    d["bqk"] = nc.dram_tensor("bqk", [128, 8], F32, kind="ExternalInput").ap()
    d["bv"] = nc.dram_tensor("bv", [128, 8, 64], F32, kind="ExternalInput").ap()
    d["wproj"] = nc.dram_tensor("wproj", [128, 4, C], F32R, kind="ExternalInput").ap()
    d["outp"] = nc.dram_tensor("outp", [T, C], F32, kind="ExternalOutput").ap()

    with tile.TileContext(nc) as tc:
        with ExitStack() as ctx, nc.allow_low_precision(reason="fp8/fp32r matmuls"):
            _body(ctx, tc, d)
    nc.compile()
    return nc


class _Kern:
    def __init__(self, ctx, tc, d):
        nc = tc.nc
        self.nc = nc
        self.tc = tc
        self.outp_d = d["outp"]

        self.const = ctx.enter_context(tc.tile_pool(name="const", bufs=1))
        self.persist = ctx.enter_context(tc.tile_pool(name="persist", bufs=1))
        self.pt_pool = ctx.enter_context(tc.tile_pool(name="pt", bufs=3))
        self.yt_pool = ctx.enter_context(tc.tile_pool(name="yt", bufs=2))
        self.out_pool = ctx.enter_context(tc.tile_pool(name="osb", bufs=3))
        self.small_pool = ctx.enter_context(tc.tile_pool(name="small", bufs=4))

        self.ps_s = ctx.enter_context(tc.tile_pool(name="ps_s", bufs=2, space="PSUM"))
        self.ps_y = ctx.enter_context(tc.tile_pool(name="ps_y", bufs=2, space="PSUM"))
        self.ps_mm = ctx.enter_context(tc.tile_pool(name="ps_mm", bufs=2, space="PSUM"))

        c = self.const
        # weights ride the ACT HWDGE ring; bf16 early-block weights first so
        # the PE can start quickly on block 0
        self.wqk16_s = c.tile([128, 8, 8, 128], BF16, tag="wqk16", name="wqk16_s")
        for kc in range(8):
            nc.scalar.dma_start(self.wqk16_s[:, kc, :, :], d["wqk16"][:, kc, :, :])
        self.wv16_s = c.tile([128, 8, 512], BF16, tag="wv16", name="wv16_s")
        nc.scalar.dma_start(self.wv16_s[:], d["wv16"][:])
        self.wqk_s = c.tile([128, 4, 2, 8, 128], FP8, tag="wqk", name="wqk_s")
        nc.scalar.dma_start(self.wqk_s[:], d["wqk"][:])
        self.wv_s = c.tile([128, 4, 2, 512], FP8, tag="wv", name="wv_s")
        nc.scalar.dma_start(self.wv_s[:], d["wv"][:])
        self.bqk_s = c.tile([128, 8], F32, tag="bqk", name="bqk_s")
        nc.scalar.dma_start(self.bqk_s[:], d["bqk"][:])
        self.bv_s = c.tile([128, 8, 64], F32, tag="bv", name="bv_s")
        nc.scalar.dma_start(self.bv_s[:], d["bv"][:])
        self.wproj_s = c.tile([128, 4, C], F32R, tag="wproj", name="wproj_s")
        for pl in range(4):
            nc.scalar.dma_start(self.wproj_s[:, pl, :], d["wproj"][:, pl, :])

        # persistent activations
        # qT chunk c = heads (2c, 2c+1) q rows d-major; kT likewise, so head
        # h's q and k share base partition 64*(h%2) (matmul operand rule)
        self.qT = self.persist.tile([128, 4, T], BF16, tag="qT", name="qT")
        self.kT = self.persist.tile([128, 4, T], BF16, tag="kT", name="kT")
        # vT8 [tok128, jp(8), i(2), h(8), v64|ones64] fp8; vT16 is the bf16
        # copy of chunks 0-3 (early tokens) used by qb0's plain-bf16 PV
        self.vT8 = self.persist.tile([128, 8, 2, 8, 128], FP8, tag="vT8", name="vT8")
        self.vT16 = self.persist.tile([128, 2, 2, 8, 128], BF16, tag="vT16", name="vT16")
        for jp in range(8):
            nc.gpsimd.memset(self.vT8[:, jp, :, :, D : 2 * D], 1.0)
        for jp in range(2):
            nc.gpsimd.memset(self.vT16[:, jp, :, :, D : 2 * D], 1.0)
        self.xt8 = self.persist.tile([128, 8, T], FP8, tag="xt8", name="xt8")
        self.xt16 = self.persist.tile([128, 8, TB], BF16, tag="xt16", name="xt16")
        self.xt_d = d["xt"]
        self.xt16_d = d["xt16"]

    def load_x(self, tb):
        nc = self.nc
        t0 = tb * TB
        if tb == 0:
            # cold start: block 0 is bf16; split the load so the PE can
            # start sooner
            for kc in range(8):
                nc.sync.dma_start(
                    self.xt16[:, kc : kc + 1, :], self.xt16_d[:, kc : kc + 1, :]
                )
        else:
            nc.sync.dma_start(
                self.xt8[:, :, t0 : t0 + TB], self.xt_d[:, :, t0 : t0 + TB]
            )

    def qk_chunk(self, tb, ch):
        """q (ch<4) or k (ch>=4) d-major rows for heads (2c, 2c+1), block tb.

        Block 0 (early tokens) runs in bf16 so qb0's few-key softmax rows
        see no fp8 noise; later blocks use fp8 DoubleRow (noise averages
        out over hundreds of keys)."""
        nc = self.nc
        t0 = tb * TB
        ps = self.ps_mm.tile([128, TB], F32, tag="mm", name=f"qk{tb}_{ch}")
        if tb == 0:
            for kc in range(8):
                nc.tensor.matmul(
                    ps[:],
                    self.wqk16_s[:, kc, ch, :],
                    self.xt16[:, kc, :],
                    start=(kc == 0),
                    stop=(kc == 7),
                )
        else:
            for kp in range(4):
                nc.tensor.matmul(
                    ps[:],
                    self.wqk_s[:, kp, :, ch, :],
                    self.xt8[:, 2 * kp : 2 * kp + 2, t0 : t0 + TB],
                    start=(kp == 0),
                    stop=(kp == 3),
                    perf_mode=DR,
                )
        dst = (self.qT if ch < 4 else self.kT)[:, ch % 4, t0 : t0 + TB]
        nc.vector.tensor_scalar_add(dst, ps[:], self.bqk_s[:, ch : ch + 1])

    def v_chunk(self, j):
        """token-major v for 128-token chunk j (all heads). Chunks 0-3
        (early tokens) run bf16 and are stored twice (bf16 for qb0's PV,
        fp8 for the later query blocks' DoubleRow PV)."""
        nc = self.nc
        t0 = j * 128
        ps = self.ps_mm.tile([128, 512], F32, tag="mm", name=f"v{j}")
        if j < 4:
            for kc in range(8):
                nc.tensor.matmul(
                    ps[:],
                    self.xt16[:, kc, t0 : t0 + 128],
                    self.wv16_s[:, kc, :],
                    start=(kc == 0),
                    stop=(kc == 7),
                )
        else:
            for kp in range(4):
                nc.tensor.matmul(
                    ps[:],
                    self.xt8[:, 2 * kp : 2 * kp + 2, t0 : t0 + 128],
                    self.wv_s[:, kp, :, :],
                    start=(kp == 0),
                    stop=(kp == 3),
                    perf_mode=DR,
                )
        psh = ps[:].rearrange("p (h d) -> p h d", h=HPC)
        dst8 = self.vT8[:, j // 2, j % 2, :, 0:D]
        nc.vector.tensor_tensor(dst8, psh, self.bv_s[:], mybir.AluOpType.add)
        if j < 4:
            dst16 = self.vT16[:, j // 2, j % 2, :, 0:D]
            nc.vector.tensor_tensor(dst16, psh, self.bv_s[:], mybir.AluOpType.add)

    def attn_block(self, h, qb):
        """Attention for head h, query block qb (QB queries)."""
        nc = self.nc
        q0 = qb * QB
        nj = (qb + 1) * (QB // 128)  # key chunks attended
        psy = self.ps_y.tile([128, QB], F32, tag="psy", name=f"psy{h}_{qb}")
        npair = nj // 2
        for p in range(npair):
            # pair-level diagonal trimming: the pair is processed at the even
            # chunk's live-query range [f0, QB); the causal mask zeroes the
            # odd chunk's above-diagonal strip inside that range
            d0 = 2 * p - (nj - 4)
            f0 = 128 * d0 if d0 > 0 else 0
            ps2 = self.ps_s.tile([128, 2, QB], F32, tag="s2", name=f"s{h}_{qb}_{p}")
            p0 = 64 * (h % 2)
            for i in range(2):
                j = 2 * p + i
                nc.tensor.matmul(
                    ps2[:, i, f0:QB],
                    self.kT[p0 : p0 + 64, h // 2, 128 * j : 128 * j + 128],
                    self.qT[p0 : p0 + 64, h // 2, q0 + f0 : q0 + QB],
                    start=True,
                    stop=True,
                )
            ptdt = BF16 if qb == 0 else FP8
            pt = self.pt_pool.tile(
                [128, 2, QB], ptdt, tag="pt16" if qb == 0 else "pt", name=f"pt{h}_{qb}_{p}"
            )
            nc.scalar.activation(
                pt[:, :, f0:QB], ps2[:, :, f0:QB], AF.Exp, scale=SCALE
            )
            for i in range(2):
                j = 2 * p + i
                d = j - (nj - 4)
                if d >= 0:
                    # mask [f0, 128(d+1)): everything above the diagonal in
                    # this plane's window (including the whole [f0, 128d)
                    # strip of the odd chunk) is zeroed by the select
                    cols = min(QB, 128 * (d + 1))
                    nc.gpsimd.affine_select(
                        out=pt[:, i, f0:cols],
                        in_=pt[:, i, f0:cols],
                        base=QB * qb - 128 * j + f0,
                        channel_multiplier=-1,
                        pattern=[[1, cols - f0]],
                        compare_op=mybir.AluOpType.is_ge,
                        fill=0.0,
                    )
            if qb == 0:
                for i in range(2):
                    nc.tensor.matmul(
                        psy[:, f0:QB],
                        self.vT16[:, p, i, h, :],
                        pt[:, i, f0:QB],
                        start=(p == 0 and i == 0),
                        stop=(p == npair - 1 and i == 1),
                    )
            else:
                nc.tensor.matmul(
                    psy[:, f0:QB],
                    self.vT8[:, p, :, h, :],
                    pt[:, :, f0:QB],
                    start=(p == 0),
                    stop=(p == npair - 1),
                    perf_mode=DR,
                )
        return psy

    def normalize(self, h, qb, psy, yt):
        """softmax-normalize head h's psy into yt plane h//2."""
        nc = self.nc
        rec = self.small_pool.tile([64, QB], F32, tag="rec", name=f"rec{h}_{qb}")
        nc.vector.reciprocal(rec[:], psy[64:128, :])
        p0 = 64 * (h % 2)
        nc.vector.tensor_tensor(
            yt[p0 : p0 + 64, h // 2, :], psy[0:64, :], rec[:], mybir.AluOpType.mult
        )

    def proj_block(self, qb, yt):
        """Projection + output DMA for query block qb."""
        nc = self.nc
        for tt in range(QB // 128):
            osb = self.out_pool.tile([128, C], F32, tag="osb", name=f"o{qb}_{tt}")
            for ncol in range(2):
                po = self.ps_mm.tile([128, 512], F32, tag="mm", name=f"po{qb}_{tt}_{ncol}")
                for pl in range(4):
                    nc.tensor.matmul(
                        po[:],
                        yt[:, pl, tt * 128 : (tt + 1) * 128],
                        self.wproj_s[:, pl, ncol * 512 : (ncol + 1) * 512],
                        start=(pl == 0),
                        stop=(pl == 3),
                    )
                nc.vector.tensor_copy(osb[:, ncol * 512 : (ncol + 1) * 512], po[:])
            r0 = qb * QB + tt * 128
            nc.sync.dma_start(self.outp_d[r0 : r0 + 128, :], osb[:])


def _body(ctx, tc, d):
    k = _Kern(ctx, tc, d)
    # filler units: qkv work for block tb = 8 qk chunks + 4 v chunks,
    # interleaved between attention heads so the PE always has ready
    # matmuls while ACT paces the softmax stream.
    k.load_x(0)
    for ch in range(8):
        k.qk_chunk(0, ch)
    for j in range(4):
        k.v_chunk(j)
    queue = []
    for tb in range(1, NTB):
        queue.append(("x", tb))
        for ch in range(8):
            queue.append(("qk", tb, ch))
        for j in range(4):
            queue.append(("v", 4 * tb + j))
    qi = 0
    pending = None
    for qb in range(NQB):
        yt = k.yt_pool.tile([128, 4, QB], F32R, tag="yt", name=f"yt{qb}")
        for h in range(HPC):
            # drain filler: qkv(qb+1) must be complete before attn(*, qb+1);
            # spread the 13 units of block qb+1 over the 8 heads of qb
            want = 2 if h % 2 == 0 else 1
            need = 0
            if qb < NQB - 1:
                need = 13 * qb + min(13, (13 * (h + 1) + 7) // 8)
            while qi < len(queue) and (qi < need or (want > 0 and qi < 13 * (qb + 1))):
                unit = queue[qi]
                if unit[0] == "x":
                    k.load_x(unit[1])
                elif unit[0] == "qk":
                    k.qk_chunk(unit[1], unit[2])
                else:
                    k.v_chunk(unit[1])
                qi += 1
                want -= 1
            psy = k.attn_block(h, qb)
            k.normalize(h, qb, psy, yt)
            if pending is not None and h == 0:
                k.proj_block(*pending)
        pending = (qb, yt)
    while qi < len(queue):
        unit = queue[qi]
        if unit[0] == "x":
            k.load_x(unit[1])
        elif unit[0] == "qk":
            k.qk_chunk(unit[1], unit[2])
        else:
            k.v_chunk(unit[1])
        qi += 1
    k.proj_block(*pending)


_CACHED_NC = None


def _get_nc():
    global _CACHED_NC
    if _CACHED_NC is None:
        _CACHED_NC = build_program()
    return _CACHED_NC


def make_in_maps(x, W_attn, b_attn, W_proj):
    x = np.ascontiguousarray(np.asarray(x, dtype=np.float32))
    W_attn = np.asarray(W_attn, dtype=np.float32)
    b_attn = np.asarray(b_attn, dtype=np.float32)
    W_proj = np.asarray(W_proj, dtype=np.float32)
    import ml_dtypes

    FP8NP = ml_dtypes.float8_e4m3
    in_maps = []
    for core in range(NCORES):
        b, g = core // 2, core % 2
        h0 = 8 * g  # first global head
        # xT [128, kc(8), T]: fp8 for blocks 1-3, bf16 copy of block 0
        xtf = x[b].T.reshape(8, 128, T).transpose(1, 0, 2)
        xt = np.ascontiguousarray(xtf.astype(FP8NP))
        xt16 = np.ascontiguousarray(xtf[:, :, 0:TB].astype(ml_dtypes.bfloat16))
        # wqk [128, kp(4), i(2), ch(8), r(128)]: chunk ch<4 = q rows of heads
        # (2ch, 2ch+1); ch>=4 = k rows of heads (2(ch-4), 2(ch-4)+1)
        wqk = np.empty((128, 4, 2, 8, 128), dtype=FP8NP)
        wqk16 = np.empty((128, 8, 8, 128), dtype=ml_dtypes.bfloat16)
        wv = np.empty((128, 4, 2, 512), dtype=FP8NP)
        wv16 = np.empty((128, 8, 512), dtype=ml_dtypes.bfloat16)
        for kp in range(4):
            for i in range(2):
                kc = 2 * kp + i  # K chunk of 128 C-rows
                rows = W_attn[128 * kc : 128 * kc + 128]
                for ch in range(8):
                    off = 0 if ch < 4 else C
                    hg = h0 + 2 * (ch % 4)
                    wcols = rows[:, off + 64 * hg : off + 64 * hg + 128]
                    wqk[:, kp, i, ch, :] = wcols
                    wqk16[:, kc, ch, :] = wcols
                vcols = rows[:, 2 * C + 512 * g : 2 * C + 512 * g + 512]
                wv[:, kp, i, :] = vcols
                wv16[:, kc, :] = vcols
        bqk = np.empty((128, 8), dtype=np.float32)
        for ch in range(8):
            off = 0 if ch < 4 else C
            hg = h0 + 2 * (ch % 4)
            bqk[:, ch] = b_attn[off + 64 * hg : off + 64 * hg + 128]
        bv = np.broadcast_to(
            b_attn[2 * C + 512 * g : 2 * C + 512 * g + 512].reshape(1, 8, 64),
            (128, 8, 64),
        )
        # wproj [128, pl(4), C]: row (p, pl) = W_proj[64*(h0 + 2pl + (p>=64)) + p%64]
        wproj = np.empty((128, 4, C), dtype=np.float32)
        for pl in range(4):
            for half in range(2):
                hg = h0 + 2 * pl + half
                wproj[64 * half : 64 * half + 64, pl, :] = W_proj[
                    64 * hg : 64 * hg + 64
                ]
        in_maps.append(
            {
                "xt": xt,
                "xt16": xt16,
                "wqk": np.ascontiguousarray(wqk),
                "wqk16": np.ascontiguousarray(wqk16),
                "wv": np.ascontiguousarray(wv),
                "wv16": np.ascontiguousarray(wv16),
                "bqk": bqk,
                "bv": np.ascontiguousarray(bv),
                "wproj": wproj,
            }
        )
    return in_maps


def run(x, W_attn, b_attn, W_proj, b_proj, trace=False, **kwargs):
    nc = _get_nc()
    in_maps = make_in_maps(x, W_attn, b_attn, W_proj)
    res = run_bass_kernel_spmd(
        nc, in_maps, core_ids=list(range(NCORES)), trace=trace, **kwargs
    )
    bp = np.asarray(b_proj, dtype=np.float32)[None, :]
    out = np.empty((B, T, C), dtype=np.float32)
    for b in range(B):
        out[b] = res.results[2 * b]["outp"] + res.results[2 * b + 1]["outp"] + bp
    return out, res


def kernel(x, W_attn, b_attn, W_proj, b_proj):
    out, _ = run(x, W_attn, b_attn, W_proj, b_proj, trace=False)
    return out


# revision 56
# speedup vs baseline: 1.3464x; 1.0999x over previous
"""Causal self-attention Trainium2 kernel.

B=4, T=2048, C=1024, H=16 heads, D=64. 8 NeuronCores, sharded
batch x head-half: core (b, g) owns batch b and heads [8g, 8g+8).
Host pre-transposes x[b] to xT8 [C, T] fp8, packs DoubleRow weight
layouts, sums the per-batch core pairs' partial proj outputs.

Device kernel (per core, SPMD):
  qkv:  q,k d-major via fp8 DoubleRow matmuls (K=2x128 per instr):
        psum chunk c = [head-c q d0:64 | k d0:64] rows; evicted (DVE,
        per-partition bias, bf16) to qkT [128, 8, T].
        v token-major (lhsT = xT chunk): psum [128 tok, 512]; evicted
        (DVE, +bias replicated, fp8) into vT8 [tok, jp, i, h, v|ones],
        the [v64|ones64] columns make the PV matmul emit softmax
        denominators on psum partitions 64:128.
  attn: per (h, qb): S^T[keys, queries] bf16 matmuls per 128-key chunk,
        processed in chunk PAIRS: exp on ACT over [128, 2, cols] psum
        -> pt fp8; causal mask via gpsimd affine_select on diagonal
        chunks; PV as ONE fp8 DoubleRow matmul per pair (contracts 256
        keys) accumulating psy [y64|den64, cols].
        normalize: DVE reciprocal (partition-shifted) + mul -> yt f32r.
  proj: partial out[tok, C] = yt.T @ W_proj rows (fp32r), evicted via
        gpsimd copy, DMA'd out; deferred one block for PE slack.
"""

import sys

sys.path.insert(0, "/opt/trn_rl_repo")

from contextlib import ExitStack

import numpy as np

import concourse.bass as bass
import concourse.mybir as mybir
import concourse.tile as tile
from concourse import bacc
from concourse.bass_utils import run_bass_kernel_spmd

F32 = mybir.dt.float32
F32R = mybir.dt.float32r
BF16 = mybir.dt.bfloat16
FP8 = mybir.dt.float8e4
AF = mybir.ActivationFunctionType
DR = mybir.MatmulPerfMode.DoubleRow

B, T, C, H, D = 4, 2048, 1024, 16, 64
NCORES = 8
HPC = H // 2  # heads per core = 8
TB = 512  # token block for qkv
NTB = T // TB  # 4
QB = 512  # query block
NQB = T // QB  # 4
SCALE = 1.0 / 8.0  # 1/sqrt(D)
# q rows are pre-scaled by SCALE/16 at eviction so S psum holds u/16
# (u = scaled logits); ACT exp uses scale=16, the DVE poly op uses psum
# directly: exp(u) ~ ((1 + x) + x^2/2)^16 at x = u/16 (rel err ~u^3/1536)
QPRE = SCALE / 16.0
EXPSCALE = 16.0
# fp8 d-split q/k staging scales: q*(1/16) * k*(1/8) = S/128 = u/16, same
# as the bf16 path; split so neither tensor hits e4m3's denormal range
QS8 = 1.0 / 16.0
KS8 = 1.0 / 8.0
# which non-diagonal S pairs (by p % 4) run their exp on the DVE
DVE_PAIRS = (3,)


def _make_exp_op():
    """Custom DVE op: out[p,n] = ((1 + in0) + in0^2 * s1)^16 (exp approx
    for pre-scaled logits). Registered into dve_ops at first use."""
    import concourse.dve_ops as dve_ops
    from concourse.dve_spec import C1, One, Spec, Src0, lower, sq
    from concourse.dve_uop import DveOpSpec

    name = "EXP16Q_ANT"
    for op in dve_ops.OPS:
        if op.name == name:
            return op

    def _ref(in0, in1, s0, s1, imm2):
        x = in0.astype(np.float32)
        s1f = float(s1.flat[0]) if isinstance(s1, np.ndarray) else float(s1)
        y = ((1.0 + x) + s1f * x * x).astype(np.float32)
        for _ in range(4):
            y = (y * y).astype(np.float32)
        return y.reshape(in0.shape)

    body = sq(sq(sq(sq((One + Src0) + sq(Src0) * C1))))
    spec = Spec(body=body, reference=_ref)
    row = max(dve_ops._SUB_OPCODE_FOR_NAME.values()) + 1
    dve_ops._SUB_OPCODE_FOR_NAME[name] = row
    ver = "v3"
    uops = lower(spec, ver=ver)
    sha = DveOpSpec(name=name, opcode=row, uops=uops, rd1_en=False).sha(ver)
    op = dve_ops.DveOp(name, spec, subdim=False, uops_sha={ver: sha})
    dve_ops.OPS.append(op)
    dve_ops.CUSTOM_DVE_SPECS[name] = spec
    return op


def build_program():
    nc = bacc.Bacc(
        "TRN2",
        target_bir_lowering=False,
        debug=False,
        num_devices=NCORES,
    )
    d = {}
    d["xt"] = nc.dram_tensor("xt", [128, 8, T], FP8, kind="ExternalInput").ap()
    d["xt16"] = nc.dram_tensor("xt16", [128, 8, TB], BF16, kind="ExternalInput").ap()
    d["wqk"] = nc.dram_tensor("wqk", [128, 4, 2, 8, 128], FP8, kind="ExternalInput").ap()
    d["wqk16"] = nc.dram_tensor(
        "wqk16", [128, 8, 8, 128], BF16, kind="ExternalInput"
    ).ap()
    d["wv"] = nc.dram_tensor("wv", [128, 4, 2, 512], FP8, kind="ExternalInput").ap()
    d["wv16"] = nc.dram_tensor("wv16", [128, 8, 512], BF16, kind="ExternalInput").ap()
    d["bqk"] = nc.dram_tensor("bqk", [128, 8], F32, kind="ExternalInput").ap()
    d["bv"] = nc.dram_tensor("bv", [128, 8, 64], F32, kind="ExternalInput").ap()
    d["wproj"] = nc.dram_tensor("wproj", [128, 4, C], F32R, kind="ExternalInput").ap()
    d["outp"] = nc.dram_tensor("outp", [T, C], F32, kind="ExternalOutput").ap()

    with tile.TileContext(nc) as tc:
        with ExitStack() as ctx, nc.allow_low_precision(reason="fp8/fp32r matmuls"):
            _body(ctx, tc, d)
    nc.compile()
    return nc


class _Kern:
    def __init__(self, ctx, tc, d):
        nc = tc.nc
        self.nc = nc
        self.tc = tc
        self.outp_d = d["outp"]
        self.exp_op = _make_exp_op()
        # optional callback emitting <=1 queued filler unit; attn_block
        # calls it after each S pair so filler matmuls land where the PE
        # is blocked on the psum double-buffer anyway
        self.drip = None

        self.const = ctx.enter_context(tc.tile_pool(name="const", bufs=1))
        self.persist = ctx.enter_context(tc.tile_pool(name="persist", bufs=1))
        self.pt_pool = ctx.enter_context(tc.tile_pool(name="pt", bufs=8))
        self.pt16_pool = ctx.enter_context(tc.tile_pool(name="pt16", bufs=2))
        self.yt_pool = ctx.enter_context(tc.tile_pool(name="yt", bufs=2))
        self.out_pool = ctx.enter_context(tc.tile_pool(name="osb", bufs=3))
        self.small_pool = ctx.enter_context(tc.tile_pool(name="small", bufs=4))

        self.ps_s = ctx.enter_context(tc.tile_pool(name="ps_s", bufs=2, space="PSUM"))
        self.ps_y = ctx.enter_context(tc.tile_pool(name="ps_y", bufs=2, space="PSUM"))
        self.ps_mm = ctx.enter_context(tc.tile_pool(name="ps_mm", bufs=2, space="PSUM"))

        c = self.const
        self.d = d
        # persistent x tiles + DMA priority order: the cold-start critical
        # path is xt16 + wqk16 (block-0 bf16 work); fp8 weights and wproj
        # are deferred into the filler queue
        self.xt8 = self.persist.tile([128, 8, T], FP8, tag="xt8", name="xt8")
        self.xt16 = self.persist.tile([128, 8, TB], BF16, tag="xt16", name="xt16")
        self.xt_d = d["xt"]
        self.xt16_d = d["xt16"]
        self.bqk_s = c.tile([128, 8], F32, tag="bqk", name="bqk_s")
        nc.scalar.dma_start(self.bqk_s[:], d["bqk"][:])
        self.bv_s = c.tile([128, 8, 64], F32, tag="bv", name="bv_s")
        nc.scalar.dma_start(self.bv_s[:], d["bv"][:])
        self.load_x(0)
        # head-0 critical path first: wqk16 chunks {0,4}, then wv16, then
        # the rest of wqk16
        self.wqk16_s = c.tile([128, 8, 8, 128], BF16, tag="wqk16", name="wqk16_s")
        self.wv16_s = c.tile([128, 8, 512], BF16, tag="wv16", name="wv16_s")
        for ch in (0, 4):
            nc.scalar.dma_start(self.wqk16_s[:, :, ch, :], d["wqk16"][:, :, ch, :])
        nc.scalar.dma_start(self.wv16_s[:], d["wv16"][:])
        for ch in (1, 5, 2, 6, 3, 7):
            nc.scalar.dma_start(self.wqk16_s[:, :, ch, :], d["wqk16"][:, :, ch, :])
        # allocated now, DMA'd from the filler queue
        self.wqk_s = c.tile([128, 4, 2, 8, 128], FP8, tag="wqk", name="wqk_s")
        self.wv_s = c.tile([128, 4, 2, 512], FP8, tag="wv", name="wv_s")
        self.wproj_s = c.tile([128, 4, C], F32R, tag="wproj", name="wproj_s")

        # persistent activations
        # bf16 q/k for block 0 only (qb0's S): chunk c = heads (2c, 2c+1)
        # d-major, so head h's q and k share base partition 64*(h%2)
        self.qT = self.persist.tile([128, 4, TB], BF16, tag="qT", name="qT")
        self.kT = self.persist.tile([128, 4, TB], BF16, tag="kT", name="kT")
        # fp8 d-split DoubleRow layout for qb>=1's S: head h lives at
        # partitions [32*(h%4), +32), planes (d 0:32 | 32:64), slot h//4
        self.qDR = self.persist.tile([128, 2, 2, T], FP8, tag="qDR", name="qDR")
        self.kDR = self.persist.tile([128, 2, 2, T], FP8, tag="kDR", name="kDR")
        self.stg_pool = ctx.enter_context(tc.tile_pool(name="stg", bufs=4))
        # vT8 [tok128, jp(8), i(2), h(8), v64|ones64] fp8; vT16 is the bf16
        # copy of chunks 0-3 (early tokens) used by qb0's plain-bf16 PV
        self.vT8 = self.persist.tile([128, 8, 2, 8, 128], FP8, tag="vT8", name="vT8")
        self.vT16 = self.persist.tile([128, 2, 2, 8, 128], BF16, tag="vT16", name="vT16")
        for jp in range(2):
            nc.gpsimd.memset(self.vT16[:, jp, :, :, D : 2 * D], 1.0)
        for jp in range(8):
            nc.gpsimd.memset(self.vT8[:, jp, :, :, D : 2 * D], 1.0)

    def load_x(self, tb):
        nc = self.nc
        t0 = tb * TB
        if tb == 0:
            # cold start: block 0 is bf16; split the load so the PE can
            # start sooner
            for kc in range(8):
                nc.sync.dma_start(
                    self.xt16[:, kc : kc + 1, :], self.xt16_d[:, kc : kc + 1, :]
                )
        else:
            nc.sync.dma_start(
                self.xt8[:, :, t0 : t0 + TB], self.xt_d[:, :, t0 : t0 + TB]
            )

    def qk_chunk(self, tb, ch):
        """q (ch<4) or k (ch>=4) d-major rows for heads (2c, 2c+1), block tb.

        Block 0 (early tokens) runs in bf16 so qb0's few-key softmax rows
        see no fp8 noise; later blocks use fp8 DoubleRow (noise averages
        out over hundreds of keys)."""
        nc = self.nc
        t0 = tb * TB
        ps = self.ps_mm.tile([128, TB], F32, tag="mm", name=f"qk{tb}_{ch}")
        if tb == 0:
            for kc in range(8):
                nc.tensor.matmul(
                    ps[:],
                    self.wqk16_s[:, kc, ch, :],
                    self.xt16[:, kc, :],
                    start=(kc == 0),
                    stop=(kc == 7),
                )
        else:
            for kp in range(4):
                nc.tensor.matmul(
                    ps[:],
                    self.wqk_s[:, kp, :, ch, :],
                    self.xt8[:, 2 * kp : 2 * kp + 2, t0 : t0 + TB],
                    start=(kp == 0),
                    stop=(kp == 3),
                    perf_mode=DR,
                )
        if tb == 0 and ch < 4:
            # qb0's bf16 q with the softmax scale folded in (see QPRE)
            nc.vector.tensor_scalar(
                self.qT[:, ch, :], ps[:],
                self.bqk_s[:, ch : ch + 1], QPRE,
                mybir.AluOpType.add, mybir.AluOpType.mult,
            )
            return
        if tb == 0:
            nc.vector.tensor_scalar_add(
                self.kT[:, ch % 4, :], ps[:], self.bqk_s[:, ch : ch + 1]
            )
        # fp8 d-split staging for the DoubleRow S path (q: blocks 1-3;
        # k: all blocks), then DMA partition-remap into qDR/kDR
        stg = self.stg_pool.tile([128, TB], FP8, tag="stg", name=f"stg{tb}_{ch}")
        nc.vector.tensor_scalar(
            stg[:], ps[:], self.bqk_s[:, ch : ch + 1], QS8 if ch < 4 else KS8,
            mybir.AluOpType.add, mybir.AluOpType.mult,
        )
        dst = self.qDR if ch < 4 else self.kDR
        for hh in (2 * (ch % 4), 2 * (ch % 4) + 1):
            g, a = hh % 4, hh // 4
            for pl in range(2):
                src0 = 64 * (hh % 2) + 32 * pl
                nc.sync.dma_start(
                    dst[32 * g : 32 * g + 32, pl, a, t0 : t0 + TB],
                    stg[src0 : src0 + 32, :],
                )

    def v_chunk(self, j):
        """token-major v for 128-token chunk j (all heads). Chunks 0-3
        (early tokens) run bf16 and are stored twice (bf16 for qb0's PV,
        fp8 for the later query blocks' DoubleRow PV)."""
        nc = self.nc
        t0 = j * 128
        ps = self.ps_mm.tile([128, 512], F32, tag="mm", name=f"v{j}")
        if j < 4:
            for kc in range(8):
                nc.tensor.matmul(
                    ps[:],
                    self.xt16[:, kc, t0 : t0 + 128],
                    self.wv16_s[:, kc, :],
                    start=(kc == 0),
                    stop=(kc == 7),
                )
        else:
            for kp in range(4):
                nc.tensor.matmul(
                    ps[:],
                    self.xt8[:, 2 * kp : 2 * kp + 2, t0 : t0 + 128],
                    self.wv_s[:, kp, :, :],
                    start=(kp == 0),
                    stop=(kp == 3),
                    perf_mode=DR,
                )
        psh = ps[:].rearrange("p (h d) -> p h d", h=HPC)
        dst8 = self.vT8[:, j // 2, j % 2, :, 0:D]
        nc.vector.tensor_tensor(dst8, psh, self.bv_s[:], mybir.AluOpType.add)
        if j < 4:
            dst16 = self.vT16[:, j // 2, j % 2, :, 0:D]
            nc.vector.tensor_tensor(dst16, psh, self.bv_s[:], mybir.AluOpType.add)

    def attn_block(self, h, qb):
        """Attention for head h, query block qb (QB queries).

        All S matmuls + exp + masks are emitted first, the PV matmuls after:
        a PV is only ready once its pair's exp lands, and emitting it inline
        would head-of-line-block the in-order PE stream (wait-queue depth 4)
        while later S matmuls are already runnable."""
        nc = self.nc
        q0 = qb * QB
        nj = (qb + 1) * (QB // 128)  # key chunks attended
        psy = self.ps_y.tile([128, QB], F32, tag="psy", name=f"psy{h}_{qb}")
        npair = nj // 2
        pts = []
        for p in range(npair):
            # pair-level diagonal trimming: the pair is processed at the even
            # chunk's live-query range [f0, QB); the causal mask zeroes the
            # odd chunk's above-diagonal strip inside that range
            d0 = 2 * p - (nj - 4)
            f0 = 128 * d0 if d0 > 0 else 0
            ps2 = self.ps_s.tile([128, 2, QB], F32, tag="s2", name=f"s{h}_{qb}_{p}")
            if qb == 0:
                p0 = 64 * (h % 2)
                for i in range(2):
                    j = 2 * p + i
                    nc.tensor.matmul(
                        ps2[:, i, f0:QB],
                        self.kT[p0 : p0 + 64, h // 2, 128 * j : 128 * j + 128],
                        self.qT[p0 : p0 + 64, h // 2, q0 + f0 : q0 + QB],
                        start=True,
                        stop=True,
                    )
            else:
                g, a = h % 4, h // 4
                for i in range(2):
                    j = 2 * p + i
                    nc.tensor.matmul(
                        ps2[:, i, f0:QB],
                        self.kDR[32 * g : 32 * g + 32, :, a, 128 * j : 128 * j + 128],
                        self.qDR[32 * g : 32 * g + 32, :, a, q0 + f0 : q0 + QB],
                        start=True,
                        stop=True,
                        perf_mode=DR,
                        tile_position=(32 * g, 0),
                    )
            if qb == 0:
                pt = self.pt16_pool.tile(
                    [128, 2, QB], BF16, tag="pt16", name=f"pt{h}_{qb}_{p}"
                )
            else:
                pt = self.pt_pool.tile(
                    [128, 2, QB], FP8, tag="pt", name=f"pt{h}_{qb}_{p}"
                )
            # the last non-diagonal pair of each block runs its exp on the
            # DVE via the custom poly op (no mask needed): it offloads the
            # ACT bottleneck exactly where the DVE would otherwise idle
            # waiting for the head's normalize
            if d0 == -2:
                nc.vector._custom_dve(
                    self.exp_op, out=pt[:], in0=ps2[:], s1=0.5
                )
            else:
                nc.scalar.activation(
                    pt[:, :, f0:QB], ps2[:, :, f0:QB], AF.Exp, scale=EXPSCALE
                )
            for i in range(2):
                j = 2 * p + i
                d = j - (nj - 4)
                if d >= 0:
                    # mask [f0, 128(d+1)): everything above the diagonal in
                    # this plane's window (including the whole [f0, 128d)
                    # strip of the odd chunk) is zeroed by the select
                    cols = min(QB, 128 * (d + 1))
                    nc.gpsimd.affine_select(
                        out=pt[:, i, f0:cols],
                        in_=pt[:, i, f0:cols],
                        base=QB * qb - 128 * j + f0,
                        channel_multiplier=-1,
                        pattern=[[1, cols - f0]],
                        compare_op=mybir.AluOpType.is_ge,
                        fill=0.0,
                    )
            pts.append((pt, f0))
        return psy, pts

    def attn_pv(self, h, qb, psy, pts):
        nc = self.nc
        npair = len(pts)
        for p, (pt, f0) in enumerate(pts):
            if qb == 0:
                for i in range(2):
                    nc.tensor.matmul(
                        psy[:, f0:QB],
                        self.vT16[:, p, i, h, :],
                        pt[:, i, f0:QB],
                        start=(p == 0 and i == 0),
                        stop=(p == npair - 1 and i == 1),
                    )
            else:
                nc.tensor.matmul(
                    psy[:, f0:QB],
                    self.vT8[:, p, :, h, :],
                    pt[:, :, f0:QB],
                    start=(p == 0),
                    stop=(p == npair - 1),
                    perf_mode=DR,
                )
        return psy

    def normalize(self, h, qb, psy, yt):
        """softmax-normalize head h's psy into yt plane h//2."""
        nc = self.nc
        rec = self.small_pool.tile([64, QB], F32, tag="rec", name=f"rec{h}_{qb}")
        nc.vector.reciprocal(rec[:], psy[64:128, :])
        p0 = 64 * (h % 2)
        nc.vector.tensor_tensor(
            yt[p0 : p0 + 64, h // 2, :], psy[0:64, :], rec[:], mybir.AluOpType.mult
        )

    def load_w8(self):
        nc = self.nc
        nc.scalar.dma_start(self.wqk_s[:], self.d["wqk"][:])
        nc.scalar.dma_start(self.wv_s[:], self.d["wv"][:])

    def load_wproj(self):
        nc = self.nc
        for pl in range(4):
            nc.scalar.dma_start(self.wproj_s[:, pl, :], self.d["wproj"][:, pl, :])

    def proj_unit(self, qb, tt, ncol, yt):
        """Projection for one (128-token, 512-col) chunk of block qb; the
        second ncol also DMAs the row block out."""
        nc = self.nc
        if ncol == 0:
            self._osb = self.out_pool.tile(
                [128, C], F32, tag="osb", name=f"o{qb}_{tt}"
            )
        osb = self._osb
        po = self.ps_mm.tile([128, 512], F32, tag="mm", name=f"po{qb}_{tt}_{ncol}")
        for pl in range(4):
            nc.tensor.matmul(
                po[:],
                yt[:, pl, tt * 128 : (tt + 1) * 128],
                self.wproj_s[:, pl, ncol * 512 : (ncol + 1) * 512],
                start=(pl == 0),
                stop=(pl == 3),
            )
        nc.vector.tensor_copy(osb[:, ncol * 512 : (ncol + 1) * 512], po[:])
        if ncol == 1:
            r0 = qb * QB + tt * 128
            nc.sync.dma_start(self.outp_d[r0 : r0 + 128, :], osb[:])


def _body(ctx, tc, d):
    k = _Kern(ctx, tc, d)

    def run_unit(unit):
        kind = unit[0]
        if kind == "x":
            k.load_x(unit[1])
        elif kind == "qk":
            k.qk_chunk(unit[1], unit[2])
        elif kind == "v":
            k.v_chunk(unit[1])
        elif kind == "w8":
            k.load_w8()
        elif kind == "wp":
            k.load_wproj()
        else:
            k.proj_unit(unit[1], unit[2], unit[3], unit[4])

    # cold-start prologue: just enough block-0 work for head 0's S/exp
    # stream, with the v chunks (needed only by head 0's PV) behind it
    k.qk_chunk(0, 0)
    k.qk_chunk(0, 4)
    psy0, pts0 = k.attn_block(0, 0)
    for j in range(4):
        k.v_chunk(j)

    # filler queue: PE-dense units (qkv for later blocks, weight loads,
    # deferred projection chunks) drained between attention heads so the PE
    # always has ready matmuls while ACT paces the softmax stream.
    queue = [("qk", 0, 1), ("qk", 0, 5), ("qk", 0, 2), ("qk", 0, 6),
             ("qk", 0, 3), ("qk", 0, 7), ("w8",), ("x", 1), ("wp",)]
    for ch in range(8):
        queue.append(("qk", 1, ch))
    for j in range(4):
        queue.append(("v", 4 + j))
    # marks[qb] = queue index that must be drained before attn(*, qb)
    marks = {0: 0, 1: len(queue)}
    for tb in range(2, NTB):
        queue.append(("x", tb))
        for ch in range(8):
            queue.append(("qk", tb, ch))
        for j in range(4):
            queue.append(("v", 4 * tb + j))
        marks[tb] = len(queue)

    qstate = {"qi": 0}

    def drip():
        i = qstate["qi"]
        if i < len(queue):
            run_unit(queue[i])
            qstate["qi"] = i + 1

    k.drip = drip
    for qb in range(NQB):
        # hard dependency: all units below marks[qb] (the previous block's
        # qkv) must be emitted before this block's attention
        while qstate["qi"] < marks.get(qb, 0):
            drip()
        yt = k.yt_pool.tile([128, 4, QB], F32R, tag="yt", name=f"yt{qb}")
        for h in range(HPC):
            # even drip: by head h of qb, progress ~(2h+1)/16 of the way
            # from marks[qb] to marks[qb+1]
            lo = marks.get(qb, 0)
            hi = marks.get(qb + 1, len(queue))
            need = lo + ((hi - lo) * (2 * h + 1) + 15) // 16
            while qstate["qi"] < min(need, len(queue)):
                drip()
            if qb == 0 and h == 0:
                psy, pts = psy0, pts0
            else:
                psy, pts = k.attn_block(h, qb)
            k.attn_pv(h, qb, psy, pts)
            k.normalize(h, qb, psy, yt)
        # projection for this block drains during the next block's heads
        if qb < NQB - 1:
            ins = max(marks[qb + 1], qstate["qi"])
            nu = 0
            for tt in range(QB // 128):
                for ncol in range(2):
                    queue.insert(ins + nu, ("pj", qb, tt, ncol, yt))
                    nu += 1
            for b2 in list(marks):
                if marks[b2] >= ins:
                    marks[b2] += nu
    while qstate["qi"] < len(queue):
        drip()
    for tt in range(QB // 128):
        for ncol in range(2):
            k.proj_unit(NQB - 1, tt, ncol, yt)


_CACHED_NC = None


def _get_nc():
    global _CACHED_NC
    if _CACHED_NC is None:
        _CACHED_NC = build_program()
    return _CACHED_NC


def make_in_maps(x, W_attn, b_attn, W_proj):
    x = np.ascontiguousarray(np.asarray(x, dtype=np.float32))
    W_attn = np.asarray(W_attn, dtype=np.float32)
    b_attn = np.asarray(b_attn, dtype=np.float32)
    W_proj = np.asarray(W_proj, dtype=np.float32)
    import ml_dtypes

    FP8NP = ml_dtypes.float8_e4m3
    in_maps = []
    for core in range(NCORES):
        b, g = core // 2, core % 2
        h0 = 8 * g  # first global head
        # xT [128, kc(8), T]: fp8 for blocks 1-3, bf16 copy of block 0
        xtf = x[b].T.reshape(8, 128, T).transpose(1, 0, 2)
        xt = np.ascontiguousarray(xtf.astype(FP8NP))
        xt16 = np.ascontiguousarray(xtf[:, :, 0:TB].astype(ml_dtypes.bfloat16))
        # wqk [128, kp(4), i(2), ch(8), r(128)]: chunk ch<4 = q rows of heads
        # (2ch, 2ch+1); ch>=4 = k rows of heads (2(ch-4), 2(ch-4)+1)
        wqk = np.empty((128, 4, 2, 8, 128), dtype=FP8NP)
        wqk16 = np.empty((128, 8, 8, 128), dtype=ml_dtypes.bfloat16)
        wv = np.empty((128, 4, 2, 512), dtype=FP8NP)
        wv16 = np.empty((128, 8, 512), dtype=ml_dtypes.bfloat16)
        for kp in range(4):
            for i in range(2):
                kc = 2 * kp + i  # K chunk of 128 C-rows
                rows = W_attn[128 * kc : 128 * kc + 128]
                for ch in range(8):
                    off = 0 if ch < 4 else C
                    hg = h0 + 2 * (ch % 4)
                    wcols = rows[:, off + 64 * hg : off + 64 * hg + 128]
                    wqk[:, kp, i, ch, :] = wcols
                    wqk16[:, kc, ch, :] = wcols
                vcols = rows[:, 2 * C + 512 * g : 2 * C + 512 * g + 512]
                wv[:, kp, i, :] = vcols
                wv16[:, kc, :] = vcols
        bqk = np.empty((128, 8), dtype=np.float32)
        for ch in range(8):
            off = 0 if ch < 4 else C
            hg = h0 + 2 * (ch % 4)
            bqk[:, ch] = b_attn[off + 64 * hg : off + 64 * hg + 128]
        bv = np.broadcast_to(
            b_attn[2 * C + 512 * g : 2 * C + 512 * g + 512].reshape(1, 8, 64),
            (128, 8, 64),
        )
        # wproj [128, pl(4), C]: row (p, pl) = W_proj[64*(h0 + 2pl + (p>=64)) + p%64]
        wproj = np.empty((128, 4, C), dtype=np.float32)
        for pl in range(4):
            for half in range(2):
                hg = h0 + 2 * pl + half
                wproj[64 * half : 64 * half + 64, pl, :] = W_proj[
                    64 * hg : 64 * hg + 64
                ]
        in_maps.append(
            {
                "xt": xt,
                "xt16": xt16,
                "wqk": np.ascontiguousarray(wqk),
                "wqk16": np.ascontiguousarray(wqk16),
                "wv": np.ascontiguousarray(wv),
                "wv16": np.ascontiguousarray(wv16),
                "bqk": bqk,
                "bv": np.ascontiguousarray(bv),
                "wproj": wproj,
            }
        )
    return in_maps


def run(x, W_attn, b_attn, W_proj, b_proj, trace=False, **kwargs):
    nc = _get_nc()
    in_maps = make_in_maps(x, W_attn, b_attn, W_proj)
    res = run_bass_kernel_spmd(
        nc, in_maps, core_ids=list(range(NCORES)), trace=trace, **kwargs
    )
    bp = np.asarray(b_proj, dtype=np.float32)[None, :]
    out = np.empty((B, T, C), dtype=np.float32)
    for b in range(B):
        out[b] = res.results[2 * b]["outp"] + res.results[2 * b + 1]["outp"] + bp
    return out, res


def kernel(x, W_attn, b_attn, W_proj, b_proj):
    out, _ = run(x, W_attn, b_attn, W_proj, b_proj, trace=False)
    return out
